# revision 25
# baseline (speedup 1.0000x reference)
"""Causal MHA (batch=4, seq=2048, dim=1024, 16 heads x 64) on 8 TRN2 NeuronCores.

Sharding: core c handles batch b = c//2 and head-group g = c%2 (8 heads).
Each core computes QKV projections for its heads, causal attention, and a
partial output projection over its 512 features. The host sums the two
partial projections per batch and transposes back.

All matmuls run in bf16 (fp32 PSUM accumulate); softmax runs without max
subtraction (logits are bounded ~|8|), with the row sums produced by an
extra ones-column appended to V during the PV matmul.
"""
import sys

sys.path.insert(0, "/opt/trn_rl_repo")

import json
import numpy as np
import ml_dtypes
from contextlib import ExitStack

import concourse.bass as bass
import concourse.tile as tile
from concourse import mybir
from concourse import bass_utils as _bu
from concourse.bass_utils import run_bass_kernel_spmd

LDW_OPT = False  # walrus ldw-opt rejects bass-emitted Ldweights outright

BF16 = mybir.dt.bfloat16
F32 = mybir.dt.float32
F32R = mybir.dt.float32r
Exp = mybir.ActivationFunctionType.Exp

DIM = 1024
SEQ = 2048
NH = 16          # total heads
HPC = 8          # heads per core
DH = 64          # head dim
SCALE = DH ** -0.5
NCORES = 8
FPC = HPC * DH   # features per core = 512
NKT = SEQ // 128   # 16 k-tiles of 128
NQC = SEQ // 512   # 4 q-chunks of 512
VSTRIDE = DH + 2   # 66: V columns per head incl. ones col + pad

_WALRUS_PATCHED = False


def _patch_walrus_wait_limit():
    """This container's walrus rejects >1 sem wait per instruction
    (CoreV3 setupSyncWait). Tile's tail drain carries one wait per live
    proc; split the extras into preceding single-wait Drain carriers at
    BIR-JSON serialization time."""
    global _WALRUS_PATCHED
    if _WALRUS_PATCHED:
        return
    _WALRUS_PATCHED = True

    if LDW_OPT:
        orig_run = _bu.run_command

        def run_patched(cmd, *a, **k):
            cmd = ["--enable-ldw-opt=true" if c == "--enable-ldw-opt=false" else c
                   for c in cmd]
            return orig_run(cmd, *a, **k)

        _bu.run_command = run_patched

    orig = bass.Bass.to_json_bytes

    def _merge_ldw_halves(insts):
        """Fold row-tiled Ldweights pairs ([64,128] at row 0 + [64,128] at
        row 64 of the same tensor) into one [128,128] load carrying both
        halves' waits."""
        out = []
        pend = None  # (index_in_out, inst) of a candidate row-0 half
        for inst in insts:
            op = inst["opcode"]
            if inst.get("engine") != "PE":
                out.append(inst)
                continue
            if op == "Ldweights" and inst.get("tile_size") == [64, 128]:
                ap = inst["ins"][0].get("ap")
                if inst.get("tile_position") == [0, 0] and ap and ap[0][1] == 64:
                    out.append(inst)
                    pend = (len(out) - 1, inst)
                    continue
                if (pend is not None
                        and inst.get("tile_position") == [64, 0] and ap
                        and ap[0][1] == 64):
                    a = pend[1]
                    aap = a["ins"][0]["ap"]
                    same = (a["ins"][0].get("memref") == inst["ins"][0].get("memref")
                            and aap[0][0] == ap[0][0] and aap[1] == ap[1]
                            and inst["ins"][0].get("offset", 0)
                            == a["ins"][0].get("offset", 0) + 64 * aap[0][0])
                    b_si = inst.get("sync_info") or {}
                    if same and not b_si.get("on_update"):
                        aap[0][1] = 128
                        a["tile_size"] = [128, 128]
                        a.setdefault("sync_info", {"on_update": [], "on_wait": []})
                        a["sync_info"].setdefault("on_wait", [])
                        a["sync_info"]["on_wait"].extend(b_si.get("on_wait") or [])
                        pend = None
                        continue
                out.append(inst)
                pend = None
            else:
                if op not in ("Matmult", "NoOp"):
                    pend = None
                out.append(inst)
        return out

    def patched(self, *a, **k):
        d = json.loads(orig(self, *a, **k))
        for f in d["functions"]:
            for bb in f["blocks"]:
                bb["instructions"] = _merge_ldw_halves(bb["instructions"])
                out = []
                last_ldw = None  # (key, still_valid)
                for inst in bb["instructions"]:
                    si = inst.get("sync_info")
                    ow = (si or {}).get("on_wait") or []
                    op = inst["opcode"]

                    def emit_carriers(waits):
                        for j, w in enumerate(waits):
                            out.append({
                                "name": f"{inst['name']}__w{j}",
                                "opcode": "NoOp",
                                "engine": inst["engine"],
                                "ins": [], "outs": [],
                                "debug": inst.get("debug", 0),
                                "sync_info": {"on_update": [], "on_wait": [w]},
                            })

                    # drop a Ldweights identical to the previous one when only
                    # Matmult/NoOp sit between (weights already resident);
                    # also fold the row-tiled [64,128]+[64,128] half-pair into
                    # the single [128,128] load emitted by _merge_ldw_halves
                    if op == "Ldweights" and inst["engine"] == "PE":
                        key = json.dumps(
                            [inst.get("ins"), inst.get("tile_position"),
                             inst.get("tile_size")], sort_keys=True)
                        if last_ldw == key and not (si or {}).get("on_update"):
                            emit_carriers(ow)
                            continue
                        last_ldw = key
                    elif inst["engine"] == "PE" and op not in ("Matmult", "NoOp"):
                        last_ldw = None

                    if len(ow) > 1:
                        emit_carriers(ow[:-1])
                        si["on_wait"] = [ow[-1]]
                    out.append(inst)
                bb["instructions"] = out
        return json.dumps(d).encode()

    bass.Bass.to_json_bytes = patched


def _act_recip2(nc, work, out, in_):
    """1/x on a row-pair (partitions 0 and 32 — sub-32 accesses must start
    at a multiple of 32) as exp(-ln(x)) — both funcs live in the
    natural_log_exp_and_others ACT table set, the same set the softmax exps
    use, so no ~1.3us ACT_TABLE_LOAD switch is ever paid (a real Reciprocal
    would switch sets every call)."""
    lrow = work.tile([33, 512], F32, tag="lrow", name="lrow")
    nc.scalar.activation(lrow[:], in_, mybir.ActivationFunctionType.Ln)
    nc.scalar.activation(out, lrow[:], Exp, scale=-1.0)


def build_kernel():
    nc = bass.Bass()
    xT = nc.declare_dram_parameter("xT", [DIM, SEQ], BF16, isOutput=False)
    wq = nc.declare_dram_parameter("wq", [DIM, FPC], BF16, isOutput=False)
    wk = nc.declare_dram_parameter("wk", [DIM, FPC], BF16, isOutput=False)
    wv = nc.declare_dram_parameter("wv", [DIM, FPC], BF16, isOutput=False)
    wo = nc.declare_dram_parameter("wo", [FPC, DIM], BF16, isOutput=False)
    # causal keep mask for the r=0 diagonal; offsets r>0 are shifted views
    msk = nc.declare_dram_parameter("msk", [128, 512], BF16, isOutput=False)
    outT = nc.declare_dram_parameter("outT", [DIM, SEQ], BF16, isOutput=True)

    with tile.TileContext(nc) as tc, ExitStack() as ctx:
        persist = ctx.enter_context(tc.tile_pool(name="persist", bufs=1))
        work = ctx.enter_context(tc.tile_pool(name="work", bufs=2))
        pt_pool = ctx.enter_context(tc.tile_pool(name="pt", bufs=1))
        ps_mm = ctx.enter_context(tc.tile_pool(name="ps_mm", bufs=2, space="PSUM"))
        ps_s = ctx.enter_context(tc.tile_pool(name="ps_s", bufs=2, space="PSUM"))
        ps_o = ctx.enter_context(tc.tile_pool(name="ps_o", bufs=2, space="PSUM"))

        # ---- load inputs. DMA-instruction issue costs ~0.65us each on any
        # queue, so batch each tensor into one instruction (xT into 4 token-
        # independent di-blocks so the first QK chains can start early).
        # Order = transfer order on the single sync queue = consumption order.
        def load_merged(name, h, d):
            t = persist.tile([128, d * h.shape[1]], BF16, tag=name, name=name)
            nc.sync.dma_start(
                t[:].rearrange("p (d f) -> p d f", d=d),
                h.ap().rearrange("(d p) f -> p d f", d=d))
            return t

        wq_t = load_merged("wq", wq, 8)
        xT_t = persist.tile([128, 8 * SEQ], BF16, tag="xT", name="xT")
        for b in range(4):
            nc.sync.dma_start(
                xT_t[:, b * 2 * SEQ:(b + 1) * 2 * SEQ].rearrange(
                    "p (d f) -> p d f", d=2),
                xT.ap()[b * 256:(b + 1) * 256, :].rearrange(
                    "(d p) f -> p d f", d=2))
        wk_t = load_merged("wk", wk, 8)
        wv_t = load_merged("wv", wv, 8)
        msk0 = persist.tile([128, 512], BF16, tag="msk0")
        nc.sync.dma_start(msk0[:], msk.ap()[:, :])
        wo_t = load_merged("wo", wo, 4)

        w_sb = {
            "wq": [wq_t[:, di * FPC:(di + 1) * FPC] for di in range(8)],
            "wk": [wk_t[:, di * FPC:(di + 1) * FPC] for di in range(8)],
            "wv": [wv_t[:, di * FPC:(di + 1) * FPC] for di in range(8)],
        }
        xT_sb = [xT_t[:, di * SEQ:(di + 1) * SEQ] for di in range(8)]
        wo_sb = [wo_t[:, fi * DIM:(fi + 1) * DIM] for fi in range(4)]
        # E[h, f] = 1 iff head-half h owns feature row f: one rank-2 matmul
        # broadcasts both halves' recip rows across their 64 partitions
        emat = persist.tile([33, 128], BF16, tag="emat")
        nc.gpsimd.memset(emat[:], 0.0)
        nc.gpsimd.memset(emat[0:1, 0:DH], 1.0)
        nc.gpsimd.memset(emat[32:33, DH:128], 1.0)
        # persistent sum-row staging: partitions 1..31 stay 1.0 forever so
        # ln/exp over [0:33] yields finite values on the unused rows (they
        # meet only zeros in emat, but NaN*0 would still poison the matmul)
        srow_p = persist.tile([33, 512], F32, tag="srow")
        nc.gpsimd.memset(srow_p[0:32, :], 1.0)

        # ---- stage B: QKV projections -----------------------------------
        # Emission order interleaves per-pair Q/K with V tile groups so the
        # attention stage (ACT exps) can start while QKV still runs on PE.
        qk_sb = {"q": [], "k": []}
        for qn in ("q", "k"):
            for fi in range(4):
                qk_sb[qn].append(
                    persist.tile([128, SEQ], BF16, tag=f"{qn}{fi}",
                                 name=f"{qn}{fi}"))
        v_sb = [persist.tile([128, HPC * VSTRIDE], BF16, tag=f"v{ti}",
                             name=f"v{ti}") for ti in range(NKT)]

        def emit_qk(qn, wn, fi):
            # Q, K in [feature, token] layout (w stationary, xT moving).
            # Two [128,512] mm-pool accumulators, two di-passes: QKV touches
            # only the mm banks, so S tiles and PV chains never inherit
            # cross-stage PSUM WAR dependencies. (The LDW dedupe still drops
            # every second weight load within a pass.)
            t = qk_sb[qn][fi]
            wt = {"wq": wq_t, "wk": wk_t}[wn]
            for p in range(2):
                ch = [ps_mm.tile([128, 512], F32, tag="mm", name="ch0"),
                      ps_mm.tile([128, 512], F32, tag="mm", name="ch1")]
                for di in range(8):
                    for tck in range(2 * p, 2 * p + 2):
                        nc.tensor.matmul(
                            ch[tck - 2 * p],
                            wt[:, di * FPC + fi * 128:di * FPC + (fi + 1) * 128],
                            xT_t[:, di * SEQ + tck * 512:di * SEQ + (tck + 1) * 512],
                            start=(di == 0), stop=(di == 7))
                for tck in range(2 * p, 2 * p + 2):
                    nc.vector.tensor_copy(
                        t[:, tck * 512:(tck + 1) * 512], ch[tck - 2 * p])

        def emit_v(ti):
            # V in [token, feature] layout (xT stationary, wv moving), strided
            # into VSTRIDE-blocks with a ones column per head
            t = v_sb[ti]
            p = ps_mm.tile([128, 512], F32, tag="mm", name="p_v")
            for di in range(8):
                nc.tensor.matmul(
                    p[:], xT_t[:, di * SEQ + ti * 128:di * SEQ + (ti + 1) * 128],
                    wv_t[:, di * FPC:(di + 1) * FPC],
                    start=(di == 0), stop=(di == 7))
            dst = t[:].rearrange("p (h c) -> p h c", h=HPC)[:, :, 0:DH]
            src = p[:].rearrange("p (h c) -> p h c", h=HPC)
            nc.vector.tensor_copy(dst, src)
            nc.gpsimd.memset(
                t[:].rearrange("p (h c) -> p h c", h=HPC)[:, :, DH:DH + 1], 1.0)

        ot_sb = [persist.tile([128, SEQ], BF16, tag=f"ot{fi}", name=f"ot{fi}")
                 for fi in range(4)]
        pts_map = {}
        po_map = {}
        rrow_map = {}

        def emit_s_tile(pr, ci, j):
            # S^T strip + exp into a pt tile for (head pair pr, q-chunk ci)
            q0 = ci * 512
            ps = ps_s.tile([128, 1024], F32, tag="s", name="ps_st")
            for half in range(2):   # head A / head B, row-tiled
                nc.tensor.matmul(
                    ps[:, half * 512:(half + 1) * 512],
                    qk_sb["k"][pr][half * 64:(half + 1) * 64,
                                   j * 128:(j + 1) * 128],
                    qk_sb["q"][pr][half * 64:(half + 1) * 64, q0:q0 + 512],
                    start=True, stop=True)
            pt = pt_pool.tile([128, 1024], BF16, tag=f"pt{j}", name="pt",
                              bufs=2)
            pts_map[(pr, ci)].append(pt)
            r = j - 4 * ci
            if r < 0:
                nc.scalar.activation(pt[:], ps[:], Exp, scale=SCALE)
            else:
                # diagonal tile: columns ql >= 128r are valid; the
                # rest must be zero (PV streams the full chunk)
                pt3 = pt[:].rearrange("p (b w) -> p b w", b=2)[:, :, 128 * r:]
                ps3 = ps[:].rearrange("p (b w) -> p b w", b=2)[:, :, 128 * r:]
                if r > 0:
                    nc.gpsimd.memset(
                        pt[:].rearrange("p (b w) -> p b w", b=2)[:, :, 0:128 * r],
                        0.0)
                nc.scalar.activation(pt3, ps3, Exp, scale=SCALE)
                # only the 128-wide diagonal band needs masking; columns past
                # it are fully valid (keep-mask would be all ones there)
                band = pt[:].rearrange("p (b w) -> p b w", b=2)[
                    :, :, 128 * r:128 * (r + 1)]
                mb = msk0[:, 0:128][:, None, :].broadcast_to([128, 2, 128])
                nc.vector.tensor_mul(band, band, mb)

        def emit_pv_step(pr, ci, j):
            # one k-tile of both halves' PV accumulation chains
            nj = 4 * ci + 4
            if j == 0:
                po_map[(pr, ci)] = [
                    ps_o.tile([DH + 1, 512], F32, tag="o", name="po")
                    for _ in range(2)]
            for half in range(2):
                h = 2 * pr + half
                nc.tensor.matmul(
                    po_map[(pr, ci)][half][:],
                    v_sb[j][:, h * VSTRIDE:h * VSTRIDE + DH + 1],
                    pts_map[(pr, ci)][j][:, half * 512:(half + 1) * 512],
                    start=(j == 0), stop=(j == nj - 1))

        def emit_recip(pr, ci):
            # gather both halves' row sums into one [2,512] tile, one LN +
            # one EXP; consumed by emit_div one phase later
            for half in range(2):
                nc.vector.tensor_copy(
                    srow_p[32 * half:32 * half + 1, :],
                    po_map[(pr, ci)][half][DH:DH + 1, :])
            rrow2 = work.tile([33, 512], BF16, tag="rrow", name="rrow")
            _act_recip2(nc, work, rrow2[:], srow_p[:])
            rrow_map[(pr, ci)] = rrow2
            pts_map.pop((pr, ci))

        def emit_div(pr, ci):
            # broadcast both recip rows across their 64-partition halves with
            # one rank-2 matmul and scale O^T into ot_sb
            q0 = ci * 512
            fi = pr
            rb_ps = ps_mm.tile([128, 512], F32, tag="mm", name="rb_ps")
            nc.tensor.matmul(rb_ps[:], emat[:], rrow_map[(pr, ci)][:],
                             start=True, stop=True)
            rb = work.tile([128, 512], BF16, tag="rb", name="rb")
            nc.vector.tensor_copy(rb[:], rb_ps[:])
            for half in range(2):
                row = half * 64
                po = po_map[(pr, ci)][half]
                nc.vector.tensor_mul(
                    ot_sb[fi][row:row + 64, q0:q0 + 512],
                    po[0:DH, :], rb[row:row + 64, :])
            po_map.pop((pr, ci))
            rrow_map.pop((pr, ci))

        def emit_phase(s_pair, pv_pair, pending_div=None):
            # Interleave S tiles of s_pair with PV chain steps of pv_pair at
            # k-tile granularity: the PE stream stays dense while ACT's exp
            # queue is consumed one phase after it is filled. PV trails the
            # S stream by LEAD tiles so pending_div's rb matmul (which gates
            # PV's PSUM buffers and itself waits on the ACT reciprocal) has
            # PE work queued ahead of it.
            sj = 4 * s_pair[1] + 4 if s_pair else 0
            pj = 4 * pv_pair[1] + 4 if pv_pair else 0
            lead = 5 if (s_pair and pv_pair) else 0
            if s_pair:
                pts_map[s_pair] = []
            if pending_div and lead == 0:
                emit_div(*pending_div)
            for jj in range(max(sj, pj + lead if pv_pair else 0)):
                if s_pair and jj < sj:
                    emit_s_tile(s_pair[0], s_pair[1], jj)
                if jj == lead and pending_div and lead > 0:
                    emit_div(*pending_div)
                if pv_pair and 0 <= jj - lead < pj:
                    emit_pv_step(pv_pair[0], pv_pair[1], jj - lead)
            if pv_pair:
                emit_recip(*pv_pair)

        def emit_proj(ci):
            # projection for chunk ci's columns (all pairs' OT rows ready)
            for ei in range(8):
                p = ps_mm.tile([128, 512], F32, tag="mm", name="p_proj")
                for fi in range(4):
                    nc.tensor.matmul(
                        p[:], wo_t[:, fi * DIM + ei * 128:fi * DIM + (ei + 1) * 128],
                        ot_sb[fi][:, ci * 512:(ci + 1) * 512],
                        start=(fi == 0), stop=(fi == 3))
                os_ = work.tile([128, 512], BF16, tag="os", name="os")
                nc.vector.tensor_copy(os_[:], p[:])
                nc.sync.dma_start(
                    outT.ap()[ei * 128:(ei + 1) * 128,
                              ci * 512:(ci + 1) * 512], os_[:])

        # Schedule: each phase runs pair pr's S tiles interleaved with pair
        # pr-1's PV chain (k-tile granular, PV trailing by 3 tiles). exp(pr)
        # completes during the phase that produces it, one phase before
        # PV(pr) consumes it; the reciprocal queued at a phase's end is
        # consumed (emit_div) a lead into the NEXT phase — the PE never
        # waits on ACT. Q/K, V and the output projection fill the remaining
        # slots (and give ACT time to catch up on its exp backlog).
        emit_qk("q", "wq", 0)
        emit_qk("k", "wk", 0)
        emit_phase((0, 0), None)
        emit_qk("q", "wq", 1)
        emit_qk("k", "wk", 1)
        for ti in range(4):
            emit_v(ti)
        emit_phase((1, 0), (0, 0))
        emit_qk("q", "wq", 2)
        emit_qk("k", "wk", 2)
        emit_phase((2, 0), (1, 0), pending_div=(0, 0))
        emit_qk("q", "wq", 3)
        emit_qk("k", "wk", 3)
        emit_phase((3, 0), (2, 0), pending_div=(1, 0))
        for ci in range(1, NQC):
            emit_phase((0, ci), (3, ci - 1), pending_div=(2, ci - 1))
            for ti in range(4 * ci, 4 * ci + 4):
                emit_v(ti)
            emit_div(3, ci - 1)
            emit_proj(ci - 1)
            emit_phase((1, ci), (0, ci))
            emit_phase((2, ci), (1, ci), pending_div=(0, ci))
            emit_phase((3, ci), (2, ci), pending_div=(1, ci))
        emit_phase(None, (3, NQC - 1), pending_div=(2, NQC - 1))
        emit_div(3, NQC - 1)
        emit_proj(NQC - 1)
    return nc


_NC = None


def _get_nc():
    global _NC
    if _NC is None:
        _patch_walrus_wait_limit()
        _NC = build_kernel()
    return _NC


def _host_masks():
    kl = np.arange(128)[:, None]
    ql = np.arange(512)[None, :]
    return (kl <= ql).astype(np.float32).astype(ml_dtypes.bfloat16)


def kernel(x, w_qkv, w_out, _trace=False, _trace_kwargs=None):
    x = np.asarray(x, dtype=np.float32)
    w_qkv = np.asarray(w_qkv, dtype=np.float32)
    w_out = np.asarray(w_out, dtype=np.float32)
    nc = _get_nc()

    msk = _host_masks()
    in_maps = []
    for c in range(NCORES):
        b, g = c // 2, c % 2
        cols = slice(g * FPC, (g + 1) * FPC)
        in_maps.append({
            "xT": np.ascontiguousarray(x[b].T).astype(ml_dtypes.bfloat16),
            "wq": w_qkv[:, 0 * DIM:1 * DIM][:, cols].astype(ml_dtypes.bfloat16),
            "wk": w_qkv[:, 1 * DIM:2 * DIM][:, cols].astype(ml_dtypes.bfloat16),
            "wv": w_qkv[:, 2 * DIM:3 * DIM][:, cols].astype(ml_dtypes.bfloat16),
            "wo": w_out[g * FPC:(g + 1) * FPC, :].astype(ml_dtypes.bfloat16),
            "msk": msk,
        })

    res = run_bass_kernel_spmd(
        nc, in_maps, core_ids=list(range(NCORES)),
        trace=_trace, **(_trace_kwargs or {}))
    out = np.empty((4, SEQ, DIM), dtype=np.float32)
    for b in range(4):
        out[b] = (res.results[2 * b]["outT"].astype(np.float32)
                  + res.results[2 * b + 1]["outT"].astype(np.float32)).T
    if _trace:
        kernel.last_results = res
    return out



# revision 26
# speedup vs baseline: 1.0216x; 1.0216x over previous
"""Causal MHA (batch=4, seq=2048, dim=1024, 16 heads x 64) on 8 TRN2 NeuronCores.

Sharding: core c handles batch b = c//2 and head-group g = c%2 (8 heads).
Each core computes QKV projections for its heads, causal attention, and a
partial output projection over its 512 features. The host sums the two
partial projections per batch and transposes back.

All matmuls run in bf16 (fp32 PSUM accumulate); softmax runs without max
subtraction (logits are bounded ~|8|), with the row sums produced by an
extra ones-column appended to V during the PV matmul.
"""
import sys

sys.path.insert(0, "/opt/trn_rl_repo")

import json
import numpy as np
import ml_dtypes
from contextlib import ExitStack

import concourse.bass as bass
import concourse.tile as tile
from concourse import mybir
from concourse import bass_utils as _bu
from concourse.bass_utils import run_bass_kernel_spmd

LDW_OPT = False  # walrus ldw-opt rejects bass-emitted Ldweights outright

BF16 = mybir.dt.bfloat16
F32 = mybir.dt.float32
F32R = mybir.dt.float32r
Exp = mybir.ActivationFunctionType.Exp

DIM = 1024
SEQ = 2048
NH = 16          # total heads
HPC = 8          # heads per core
DH = 64          # head dim
SCALE = DH ** -0.5
NCORES = 8
FPC = HPC * DH   # features per core = 512
NKT = SEQ // 128   # 16 k-tiles of 128
NQC = SEQ // 512   # 4 q-chunks of 512
VSTRIDE = DH + 2   # 66: V columns per head incl. ones col + pad

_WALRUS_PATCHED = False


def _patch_walrus_wait_limit():
    """This container's walrus rejects >1 sem wait per instruction
    (CoreV3 setupSyncWait). Tile's tail drain carries one wait per live
    proc; split the extras into preceding single-wait Drain carriers at
    BIR-JSON serialization time."""
    global _WALRUS_PATCHED
    if _WALRUS_PATCHED:
        return
    _WALRUS_PATCHED = True

    if LDW_OPT:
        orig_run = _bu.run_command

        def run_patched(cmd, *a, **k):
            cmd = ["--enable-ldw-opt=true" if c == "--enable-ldw-opt=false" else c
                   for c in cmd]
            return orig_run(cmd, *a, **k)

        _bu.run_command = run_patched

    orig = bass.Bass.to_json_bytes

    def _merge_ldw_halves(insts):
        """Fold row-tiled Ldweights pairs ([64,128] at row 0 + [64,128] at
        row 64 of the same tensor) into one [128,128] load carrying both
        halves' waits."""
        out = []
        pend = None  # (index_in_out, inst) of a candidate row-0 half
        for inst in insts:
            op = inst["opcode"]
            if inst.get("engine") != "PE":
                out.append(inst)
                continue
            if op == "Ldweights" and inst.get("tile_size") == [64, 128]:
                ap = inst["ins"][0].get("ap")
                if inst.get("tile_position") == [0, 0] and ap and ap[0][1] == 64:
                    out.append(inst)
                    pend = (len(out) - 1, inst)
                    continue
                if (pend is not None
                        and inst.get("tile_position") == [64, 0] and ap
                        and ap[0][1] == 64):
                    a = pend[1]
                    aap = a["ins"][0]["ap"]
                    same = (a["ins"][0].get("memref") == inst["ins"][0].get("memref")
                            and aap[0][0] == ap[0][0] and aap[1] == ap[1]
                            and inst["ins"][0].get("offset", 0)
                            == a["ins"][0].get("offset", 0) + 64 * aap[0][0])
                    b_si = inst.get("sync_info") or {}
                    if same and not b_si.get("on_update"):
                        aap[0][1] = 128
                        a["tile_size"] = [128, 128]
                        a.setdefault("sync_info", {"on_update": [], "on_wait": []})
                        a["sync_info"].setdefault("on_wait", [])
                        a["sync_info"]["on_wait"].extend(b_si.get("on_wait") or [])
                        pend = None
                        continue
                out.append(inst)
                pend = None
            else:
                if op not in ("Matmult", "NoOp"):
                    pend = None
                out.append(inst)
        return out

    def patched(self, *a, **k):
        d = json.loads(orig(self, *a, **k))
        for f in d["functions"]:
            for bb in f["blocks"]:
                bb["instructions"] = _merge_ldw_halves(bb["instructions"])
                out = []
                last_ldw = None  # (key, still_valid)
                for inst in bb["instructions"]:
                    si = inst.get("sync_info")
                    ow = (si or {}).get("on_wait") or []
                    op = inst["opcode"]

                    def emit_carriers(waits):
                        for j, w in enumerate(waits):
                            out.append({
                                "name": f"{inst['name']}__w{j}",
                                "opcode": "NoOp",
                                "engine": inst["engine"],
                                "ins": [], "outs": [],
                                "debug": inst.get("debug", 0),
                                "sync_info": {"on_update": [], "on_wait": [w]},
                            })

                    # drop a Ldweights identical to the previous one when only
                    # Matmult/NoOp sit between (weights already resident);
                    # also fold the row-tiled [64,128]+[64,128] half-pair into
                    # the single [128,128] load emitted by _merge_ldw_halves
                    if op == "Ldweights" and inst["engine"] == "PE":
                        key = json.dumps(
                            [inst.get("ins"), inst.get("tile_position"),
                             inst.get("tile_size")], sort_keys=True)
                        if last_ldw == key and not (si or {}).get("on_update"):
                            emit_carriers(ow)
                            continue
                        last_ldw = key
                    elif inst["engine"] == "PE" and op not in ("Matmult", "NoOp"):
                        last_ldw = None

                    if len(ow) > 1:
                        emit_carriers(ow[:-1])
                        si["on_wait"] = [ow[-1]]
                    out.append(inst)
                bb["instructions"] = out
        return json.dumps(d).encode()

    bass.Bass.to_json_bytes = patched


def _act_recip2(nc, work, out, in_):
    """1/x on a row-pair (partitions 0 and 32 — sub-32 accesses must start
    at a multiple of 32) as exp(-ln(x)) — both funcs live in the
    natural_log_exp_and_others ACT table set, the same set the softmax exps
    use, so no ~1.3us ACT_TABLE_LOAD switch is ever paid (a real Reciprocal
    would switch sets every call)."""
    lrow = work.tile([33, 512], F32, tag="lrow", name="lrow")
    nc.scalar.activation(lrow[:], in_, mybir.ActivationFunctionType.Ln)
    nc.scalar.activation(out, lrow[:], Exp, scale=-1.0)


def build_kernel():
    nc = bass.Bass()
    xT = nc.declare_dram_parameter("xT", [DIM, SEQ], BF16, isOutput=False)
    wq = nc.declare_dram_parameter("wq", [DIM, FPC], BF16, isOutput=False)
    wk = nc.declare_dram_parameter("wk", [DIM, FPC], BF16, isOutput=False)
    wv = nc.declare_dram_parameter("wv", [DIM, FPC], BF16, isOutput=False)
    wo = nc.declare_dram_parameter("wo", [FPC, DIM], BF16, isOutput=False)
    # causal keep mask for the r=0 diagonal; offsets r>0 are shifted views
    msk = nc.declare_dram_parameter("msk", [128, 512], BF16, isOutput=False)
    outT = nc.declare_dram_parameter("outT", [DIM, SEQ], BF16, isOutput=True)

    with tile.TileContext(nc) as tc, ExitStack() as ctx:
        persist = ctx.enter_context(tc.tile_pool(name="persist", bufs=1))
        work = ctx.enter_context(tc.tile_pool(name="work", bufs=2))
        pt_pool = ctx.enter_context(tc.tile_pool(name="pt", bufs=1))
        ps_mm = ctx.enter_context(tc.tile_pool(name="ps_mm", bufs=2, space="PSUM"))
        ps_s = ctx.enter_context(tc.tile_pool(name="ps_s", bufs=2, space="PSUM"))
        ps_o = ctx.enter_context(tc.tile_pool(name="ps_o", bufs=2, space="PSUM"))

        # ---- load inputs. DMA-instruction issue costs ~0.65us each on any
        # queue, so batch each tensor into one instruction (xT into 4 token-
        # independent di-blocks so the first QK chains can start early).
        # Order = transfer order on the single sync queue = consumption order.
        def load_merged(name, h, d):
            t = persist.tile([128, d * h.shape[1]], BF16, tag=name, name=name)
            nc.sync.dma_start(
                t[:].rearrange("p (d f) -> p d f", d=d),
                h.ap().rearrange("(d p) f -> p d f", d=d))
            return t

        wq_t = load_merged("wq", wq, 8)
        xT_t = persist.tile([128, 8 * SEQ], BF16, tag="xT", name="xT")
        for b in range(4):
            # token-block b across all 8 di-slices: the first QK chain (one
            # 512-token column) can start once block 0 lands
            nc.sync.dma_start(
                xT_t[:].rearrange("p (d f) -> p d f", d=8)[
                    :, :, b * 512:(b + 1) * 512],
                xT.ap()[:, b * 512:(b + 1) * 512].rearrange(
                    "(d p) f -> p d f", d=8))
        wk_t = load_merged("wk", wk, 8)
        wv_t = load_merged("wv", wv, 8)
        msk0 = persist.tile([128, 512], BF16, tag="msk0")
        nc.sync.dma_start(msk0[:], msk.ap()[:, :])
        wo_t = load_merged("wo", wo, 4)

        w_sb = {
            "wq": [wq_t[:, di * FPC:(di + 1) * FPC] for di in range(8)],
            "wk": [wk_t[:, di * FPC:(di + 1) * FPC] for di in range(8)],
            "wv": [wv_t[:, di * FPC:(di + 1) * FPC] for di in range(8)],
        }
        xT_sb = [xT_t[:, di * SEQ:(di + 1) * SEQ] for di in range(8)]
        wo_sb = [wo_t[:, fi * DIM:(fi + 1) * DIM] for fi in range(4)]
        # E[h, f] = 1 iff head-half h owns feature row f: one rank-2 matmul
        # broadcasts both halves' recip rows across their 64 partitions
        emat = persist.tile([33, 128], BF16, tag="emat")
        nc.gpsimd.memset(emat[:], 0.0)
        nc.gpsimd.memset(emat[0:1, 0:DH], 1.0)
        nc.gpsimd.memset(emat[32:33, DH:128], 1.0)
        # persistent sum-row staging: partitions 1..31 stay 1.0 forever so
        # ln/exp over [0:33] yields finite values on the unused rows (they
        # meet only zeros in emat, but NaN*0 would still poison the matmul)
        srow_p = persist.tile([33, 512], F32, tag="srow")
        nc.gpsimd.memset(srow_p[0:32, :], 1.0)

        # ---- stage B: QKV projections -----------------------------------
        # Emission order interleaves per-pair Q/K with V tile groups so the
        # attention stage (ACT exps) can start while QKV still runs on PE.
        qk_sb = {"q": [], "k": []}
        for qn in ("q", "k"):
            for fi in range(4):
                qk_sb[qn].append(
                    persist.tile([128, SEQ], BF16, tag=f"{qn}{fi}",
                                 name=f"{qn}{fi}"))
        v_sb = [persist.tile([128, HPC * VSTRIDE], BF16, tag=f"v{ti}",
                             name=f"v{ti}") for ti in range(NKT)]

        def emit_qk(qn, wn, fi, tcks=range(4)):
            # Q, K in [feature, token] layout (w stationary, xT moving).
            # 4 accumulators: 2 mm banks + the two halves of one s-tile.
            t = qk_sb[qn][fi]
            wt = {"wq": wq_t, "wk": wk_t}[wn]
            st = ps_s.tile([128, 1024], F32, tag="s", name="ch_s")
            ch = [ps_mm.tile([128, 512], F32, tag="mm", name="ch0"),
                  ps_mm.tile([128, 512], F32, tag="mm", name="ch1"),
                  st[:, 0:512], st[:, 512:1024]]
            for k, tck in enumerate(tcks):
                for di in range(8):
                    nc.tensor.matmul(
                        ch[k],
                        wt[:, di * FPC + fi * 128:di * FPC + (fi + 1) * 128],
                        xT_t[:, di * SEQ + tck * 512:di * SEQ + (tck + 1) * 512],
                        start=(di == 0), stop=(di == 7))
            for k, tck in enumerate(tcks):
                nc.vector.tensor_copy(t[:, tck * 512:(tck + 1) * 512], ch[k])

        def emit_v(ti):
            # V in [token, feature] layout (xT stationary, wv moving), strided
            # into VSTRIDE-blocks with a ones column per head
            t = v_sb[ti]
            p = ps_mm.tile([128, 512], F32, tag="mm", name="p_v")
            for di in range(8):
                nc.tensor.matmul(
                    p[:], xT_t[:, di * SEQ + ti * 128:di * SEQ + (ti + 1) * 128],
                    wv_t[:, di * FPC:(di + 1) * FPC],
                    start=(di == 0), stop=(di == 7))
            dst = t[:].rearrange("p (h c) -> p h c", h=HPC)[:, :, 0:DH]
            src = p[:].rearrange("p (h c) -> p h c", h=HPC)
            nc.vector.tensor_copy(dst, src)
            nc.gpsimd.memset(
                t[:].rearrange("p (h c) -> p h c", h=HPC)[:, :, DH:DH + 1], 1.0)

        ot_sb = [persist.tile([128, SEQ], BF16, tag=f"ot{fi}", name=f"ot{fi}")
                 for fi in range(4)]
        pts_map = {}
        po_map = {}
        rrow_map = {}

        def emit_s_tile(pr, ci, j):
            # S^T strip + exp into a pt tile for (head pair pr, q-chunk ci)
            q0 = ci * 512
            ps = ps_s.tile([128, 1024], F32, tag="s", name="ps_st")
            for half in range(2):   # head A / head B, row-tiled
                nc.tensor.matmul(
                    ps[:, half * 512:(half + 1) * 512],
                    qk_sb["k"][pr][half * 64:(half + 1) * 64,
                                   j * 128:(j + 1) * 128],
                    qk_sb["q"][pr][half * 64:(half + 1) * 64, q0:q0 + 512],
                    start=True, stop=True)
            pt = pt_pool.tile([128, 1024], BF16, tag=f"pt{j}", name="pt",
                              bufs=2)
            pts_map[(pr, ci)].append(pt)
            r = j - 4 * ci
            if r < 0:
                nc.scalar.activation(pt[:], ps[:], Exp, scale=SCALE)
            else:
                # diagonal tile: columns ql >= 128r are valid; the
                # rest must be zero (PV streams the full chunk)
                pt3 = pt[:].rearrange("p (b w) -> p b w", b=2)[:, :, 128 * r:]
                ps3 = ps[:].rearrange("p (b w) -> p b w", b=2)[:, :, 128 * r:]
                if r > 0:
                    nc.gpsimd.memset(
                        pt[:].rearrange("p (b w) -> p b w", b=2)[:, :, 0:128 * r],
                        0.0)
                nc.scalar.activation(pt3, ps3, Exp, scale=SCALE)
                # only the 128-wide diagonal band needs masking; columns past
                # it are fully valid (keep-mask would be all ones there)
                band = pt[:].rearrange("p (b w) -> p b w", b=2)[
                    :, :, 128 * r:128 * (r + 1)]
                mb = msk0[:, 0:128][:, None, :].broadcast_to([128, 2, 128])
                nc.gpsimd.tensor_mul(band, band, mb)

        def emit_pv_step(pr, ci, j):
            # one k-tile of both halves' PV accumulation chains
            nj = 4 * ci + 4
            if j == 0:
                po_map[(pr, ci)] = [
                    ps_o.tile([DH + 1, 512], F32, tag="o", name="po")
                    for _ in range(2)]
            for half in range(2):
                h = 2 * pr + half
                nc.tensor.matmul(
                    po_map[(pr, ci)][half][:],
                    v_sb[j][:, h * VSTRIDE:h * VSTRIDE + DH + 1],
                    pts_map[(pr, ci)][j][:, half * 512:(half + 1) * 512],
                    start=(j == 0), stop=(j == nj - 1))

        def emit_recip(pr, ci):
            # gather both halves' row sums into one [2,512] tile, one LN +
            # one EXP; consumed by emit_div one phase later
            for half in range(2):
                nc.vector.tensor_copy(
                    srow_p[32 * half:32 * half + 1, :],
                    po_map[(pr, ci)][half][DH:DH + 1, :])
            rrow2 = work.tile([33, 512], BF16, tag="rrow", name="rrow")
            _act_recip2(nc, work, rrow2[:], srow_p[:])
            rrow_map[(pr, ci)] = rrow2
            pts_map.pop((pr, ci))

        def emit_div(pr, ci):
            # broadcast both recip rows across their 64-partition halves with
            # one rank-2 matmul and scale O^T into ot_sb
            q0 = ci * 512
            fi = pr
            rb_ps = ps_mm.tile([128, 512], F32, tag="mm", name="rb_ps")
            nc.tensor.matmul(rb_ps[:], emat[:], rrow_map[(pr, ci)][:],
                             start=True, stop=True)
            rb = work.tile([128, 512], BF16, tag="rb", name="rb")
            nc.vector.tensor_copy(rb[:], rb_ps[:])
            for half in range(2):
                row = half * 64
                po = po_map[(pr, ci)][half]
                nc.vector.tensor_mul(
                    ot_sb[fi][row:row + 64, q0:q0 + 512],
                    po[0:DH, :], rb[row:row + 64, :])
            po_map.pop((pr, ci))
            rrow_map.pop((pr, ci))

        def emit_phase(s_pair, pv_pair, pending_div=None):
            # Interleave S tiles of s_pair with PV chain steps of pv_pair at
            # k-tile granularity: the PE stream stays dense while ACT's exp
            # queue is consumed one phase after it is filled. PV trails the
            # S stream by LEAD tiles so pending_div's rb matmul (which gates
            # PV's PSUM buffers and itself waits on the ACT reciprocal) has
            # PE work queued ahead of it.
            sj = 4 * s_pair[1] + 4 if s_pair else 0
            pj = 4 * pv_pair[1] + 4 if pv_pair else 0
            lead = 5 if (s_pair and pv_pair) else 0
            if s_pair:
                pts_map[s_pair] = []
            if pending_div and lead == 0:
                emit_div(*pending_div)
            for jj in range(max(sj, pj + lead if pv_pair else 0)):
                if s_pair and jj < sj:
                    emit_s_tile(s_pair[0], s_pair[1], jj)
                if jj == lead and pending_div and lead > 0:
                    emit_div(*pending_div)
                if pv_pair and 0 <= jj - lead < pj:
                    emit_pv_step(pv_pair[0], pv_pair[1], jj - lead)
            if pv_pair:
                emit_recip(*pv_pair)

        def emit_proj(ci):
            # projection for chunk ci's columns (all pairs' OT rows ready)
            for ei in range(8):
                p = ps_mm.tile([128, 512], F32, tag="mm", name="p_proj")
                for fi in range(4):
                    nc.tensor.matmul(
                        p[:], wo_t[:, fi * DIM + ei * 128:fi * DIM + (ei + 1) * 128],
                        ot_sb[fi][:, ci * 512:(ci + 1) * 512],
                        start=(fi == 0), stop=(fi == 3))
                os_ = work.tile([128, 512], BF16, tag="os", name="os")
                nc.vector.tensor_copy(os_[:], p[:])
                nc.sync.dma_start(
                    outT.ap()[ei * 128:(ei + 1) * 128,
                              ci * 512:(ci + 1) * 512], os_[:])

        # Schedule: each phase runs pair pr's S tiles interleaved with pair
        # pr-1's PV chain (k-tile granular, PV trailing by 3 tiles). exp(pr)
        # completes during the phase that produces it, one phase before
        # PV(pr) consumes it; the reciprocal queued at a phase's end is
        # consumed (emit_div) a lead into the NEXT phase — the PE never
        # waits on ACT. Q/K, V and the output projection fill the remaining
        # slots (and give ACT time to catch up on its exp backlog).
        emit_qk("q", "wq", 0)
        emit_qk("k", "wk", 0)
        emit_phase((0, 0), None)
        emit_qk("q", "wq", 1)
        emit_qk("k", "wk", 1)
        for ti in range(4):
            emit_v(ti)
        emit_phase((1, 0), (0, 0))
        emit_qk("q", "wq", 2)
        emit_qk("k", "wk", 2)
        emit_phase((2, 0), (1, 0), pending_div=(0, 0))
        emit_qk("q", "wq", 3)
        emit_qk("k", "wk", 3)
        emit_phase((3, 0), (2, 0), pending_div=(1, 0))
        for ci in range(1, NQC):
            emit_phase((0, ci), (3, ci - 1), pending_div=(2, ci - 1))
            for ti in range(4 * ci, 4 * ci + 4):
                emit_v(ti)
            emit_div(3, ci - 1)
            emit_proj(ci - 1)
            emit_phase((1, ci), (0, ci))
            emit_phase((2, ci), (1, ci), pending_div=(0, ci))
            emit_phase((3, ci), (2, ci), pending_div=(1, ci))
        emit_phase(None, (3, NQC - 1), pending_div=(2, NQC - 1))
        emit_div(3, NQC - 1)
        emit_proj(NQC - 1)
    return nc


_NC = None


def _get_nc():
    global _NC
    if _NC is None:
        _patch_walrus_wait_limit()
        _NC = build_kernel()
    return _NC


def _host_masks():
    kl = np.arange(128)[:, None]
    ql = np.arange(512)[None, :]
    return (kl <= ql).astype(np.float32).astype(ml_dtypes.bfloat16)


def kernel(x, w_qkv, w_out, _trace=False, _trace_kwargs=None):
    x = np.asarray(x, dtype=np.float32)
    w_qkv = np.asarray(w_qkv, dtype=np.float32)
    w_out = np.asarray(w_out, dtype=np.float32)
    nc = _get_nc()

    msk = _host_masks()
    in_maps = []
    for c in range(NCORES):
        b, g = c // 2, c % 2
        cols = slice(g * FPC, (g + 1) * FPC)
        in_maps.append({
            "xT": np.ascontiguousarray(x[b].T).astype(ml_dtypes.bfloat16),
            "wq": w_qkv[:, 0 * DIM:1 * DIM][:, cols].astype(ml_dtypes.bfloat16),
            "wk": w_qkv[:, 1 * DIM:2 * DIM][:, cols].astype(ml_dtypes.bfloat16),
            "wv": w_qkv[:, 2 * DIM:3 * DIM][:, cols].astype(ml_dtypes.bfloat16),
            "wo": w_out[g * FPC:(g + 1) * FPC, :].astype(ml_dtypes.bfloat16),
            "msk": msk,
        })

    res = run_bass_kernel_spmd(
        nc, in_maps, core_ids=list(range(NCORES)),
        trace=_trace, **(_trace_kwargs or {}))
    out = np.empty((4, SEQ, DIM), dtype=np.float32)
    for b in range(4):
        out[b] = (res.results[2 * b]["outT"].astype(np.float32)
                  + res.results[2 * b + 1]["outT"].astype(np.float32)).T
    if _trace:
        kernel.last_results = res
    return out



# revision 27
# speedup vs baseline: 1.0221x; 1.0005x over previous
"""Causal MHA (batch=4, seq=2048, dim=1024, 16 heads x 64) on 8 TRN2 NeuronCores.

Sharding: core c handles batch b = c//2 and head-group g = c%2 (8 heads).
Each core computes QKV projections for its heads, causal attention, and a
partial output projection over its 512 features. The host sums the two
partial projections per batch and transposes back.

All matmuls run in bf16 (fp32 PSUM accumulate); softmax runs without max
subtraction (logits are bounded ~|8|), with the row sums produced by an
extra ones-column appended to V during the PV matmul.
"""
import sys

sys.path.insert(0, "/opt/trn_rl_repo")

import json
import numpy as np
import ml_dtypes
from contextlib import ExitStack

import concourse.bass as bass
import concourse.tile as tile
from concourse import mybir
from concourse import bass_utils as _bu
from concourse.bass_utils import run_bass_kernel_spmd

LDW_OPT = False  # walrus ldw-opt rejects bass-emitted Ldweights outright

BF16 = mybir.dt.bfloat16
F32 = mybir.dt.float32
F32R = mybir.dt.float32r
Exp = mybir.ActivationFunctionType.Exp

DIM = 1024
SEQ = 2048
NH = 16          # total heads
HPC = 8          # heads per core
DH = 64          # head dim
SCALE = DH ** -0.5
NCORES = 8
FPC = HPC * DH   # features per core = 512
NKT = SEQ // 128   # 16 k-tiles of 128
NQC = SEQ // 512   # 4 q-chunks of 512
VSTRIDE = DH + 2   # 66: V columns per head incl. ones col + pad

_WALRUS_PATCHED = False


def _patch_walrus_wait_limit():
    """This container's walrus rejects >1 sem wait per instruction
    (CoreV3 setupSyncWait). Tile's tail drain carries one wait per live
    proc; split the extras into preceding single-wait Drain carriers at
    BIR-JSON serialization time."""
    global _WALRUS_PATCHED
    if _WALRUS_PATCHED:
        return
    _WALRUS_PATCHED = True

    if LDW_OPT:
        orig_run = _bu.run_command

        def run_patched(cmd, *a, **k):
            cmd = ["--enable-ldw-opt=true" if c == "--enable-ldw-opt=false" else c
                   for c in cmd]
            return orig_run(cmd, *a, **k)

        _bu.run_command = run_patched

    orig = bass.Bass.to_json_bytes

    def _merge_ldw_halves(insts):
        """Fold row-tiled Ldweights pairs ([64,128] at row 0 + [64,128] at
        row 64 of the same tensor) into one [128,128] load carrying both
        halves' waits."""
        out = []
        pend = None  # (index_in_out, inst) of a candidate row-0 half
        for inst in insts:
            op = inst["opcode"]
            if inst.get("engine") != "PE":
                out.append(inst)
                continue
            if op == "Ldweights" and inst.get("tile_size") == [64, 128]:
                ap = inst["ins"][0].get("ap")
                if inst.get("tile_position") == [0, 0] and ap and ap[0][1] == 64:
                    out.append(inst)
                    pend = (len(out) - 1, inst)
                    continue
                if (pend is not None
                        and inst.get("tile_position") == [64, 0] and ap
                        and ap[0][1] == 64):
                    a = pend[1]
                    aap = a["ins"][0]["ap"]
                    same = (a["ins"][0].get("memref") == inst["ins"][0].get("memref")
                            and aap[0][0] == ap[0][0] and aap[1] == ap[1]
                            and inst["ins"][0].get("offset", 0)
                            == a["ins"][0].get("offset", 0) + 64 * aap[0][0])
                    b_si = inst.get("sync_info") or {}
                    if same and not b_si.get("on_update"):
                        aap[0][1] = 128
                        a["tile_size"] = [128, 128]
                        a.setdefault("sync_info", {"on_update": [], "on_wait": []})
                        a["sync_info"].setdefault("on_wait", [])
                        a["sync_info"]["on_wait"].extend(b_si.get("on_wait") or [])
                        pend = None
                        continue
                out.append(inst)
                pend = None
            else:
                if op not in ("Matmult", "NoOp"):
                    pend = None
                out.append(inst)
        return out

    def patched(self, *a, **k):
        d = json.loads(orig(self, *a, **k))
        for f in d["functions"]:
            for bb in f["blocks"]:
                bb["instructions"] = _merge_ldw_halves(bb["instructions"])
                out = []
                last_ldw = None  # (key, still_valid)
                for inst in bb["instructions"]:
                    si = inst.get("sync_info")
                    ow = (si or {}).get("on_wait") or []
                    op = inst["opcode"]

                    def emit_carriers(waits):
                        for j, w in enumerate(waits):
                            out.append({
                                "name": f"{inst['name']}__w{j}",
                                "opcode": "NoOp",
                                "engine": inst["engine"],
                                "ins": [], "outs": [],
                                "debug": inst.get("debug", 0),
                                "sync_info": {"on_update": [], "on_wait": [w]},
                            })

                    # drop a Ldweights identical to the previous one when only
                    # Matmult/NoOp sit between (weights already resident);
                    # also fold the row-tiled [64,128]+[64,128] half-pair into
                    # the single [128,128] load emitted by _merge_ldw_halves
                    if op == "Ldweights" and inst["engine"] == "PE":
                        key = json.dumps(
                            [inst.get("ins"), inst.get("tile_position"),
                             inst.get("tile_size")], sort_keys=True)
                        if last_ldw == key and not (si or {}).get("on_update"):
                            emit_carriers(ow)
                            continue
                        last_ldw = key
                    elif inst["engine"] == "PE" and op not in ("Matmult", "NoOp"):
                        last_ldw = None

                    if len(ow) > 1:
                        emit_carriers(ow[:-1])
                        si["on_wait"] = [ow[-1]]
                    out.append(inst)
                bb["instructions"] = out
        return json.dumps(d).encode()

    bass.Bass.to_json_bytes = patched


def _act_recip2(nc, work, out, in_):
    """1/x on a row-pair (partitions 0 and 32 — sub-32 accesses must start
    at a multiple of 32) as exp(-ln(x)) — both funcs live in the
    natural_log_exp_and_others ACT table set, the same set the softmax exps
    use, so no ~1.3us ACT_TABLE_LOAD switch is ever paid (a real Reciprocal
    would switch sets every call)."""
    lrow = work.tile([33, 512], F32, tag="lrow", name="lrow")
    nc.scalar.activation(lrow[:], in_, mybir.ActivationFunctionType.Ln)
    nc.scalar.activation(out, lrow[:], Exp, scale=-1.0)


def build_kernel():
    nc = bass.Bass()
    xT = nc.declare_dram_parameter("xT", [DIM, SEQ], BF16, isOutput=False)
    wq = nc.declare_dram_parameter("wq", [DIM, FPC], BF16, isOutput=False)
    wk = nc.declare_dram_parameter("wk", [DIM, FPC], BF16, isOutput=False)
    wv = nc.declare_dram_parameter("wv", [DIM, FPC], BF16, isOutput=False)
    wo = nc.declare_dram_parameter("wo", [FPC, DIM], BF16, isOutput=False)
    # causal keep mask for the r=0 diagonal; offsets r>0 are shifted views
    msk = nc.declare_dram_parameter("msk", [128, 512], BF16, isOutput=False)
    outT = nc.declare_dram_parameter("outT", [DIM, SEQ], BF16, isOutput=True)

    with tile.TileContext(nc) as tc, ExitStack() as ctx:
        persist = ctx.enter_context(tc.tile_pool(name="persist", bufs=1))
        work = ctx.enter_context(tc.tile_pool(name="work", bufs=2))
        pt_pool = ctx.enter_context(tc.tile_pool(name="pt", bufs=1))
        ps_mm = ctx.enter_context(tc.tile_pool(name="ps_mm", bufs=2, space="PSUM"))
        ps_s = ctx.enter_context(tc.tile_pool(name="ps_s", bufs=2, space="PSUM"))
        ps_o = ctx.enter_context(tc.tile_pool(name="ps_o", bufs=2, space="PSUM"))

        # ---- load inputs. DMA-instruction issue costs ~0.65us each on any
        # queue, so batch each tensor into one instruction (xT into 4 token-
        # independent di-blocks so the first QK chains can start early).
        # Order = transfer order on the single sync queue = consumption order.
        def load_merged(name, h, d):
            t = persist.tile([128, d * h.shape[1]], BF16, tag=name, name=name)
            nc.sync.dma_start(
                t[:].rearrange("p (d f) -> p d f", d=d),
                h.ap().rearrange("(d p) f -> p d f", d=d))
            return t

        wq_t = load_merged("wq", wq, 8)
        xT_t = persist.tile([128, 8 * SEQ], BF16, tag="xT", name="xT")
        for b in range(4):
            nc.sync.dma_start(
                xT_t[:, b * 2 * SEQ:(b + 1) * 2 * SEQ].rearrange(
                    "p (d f) -> p d f", d=2),
                xT.ap()[b * 256:(b + 1) * 256, :].rearrange(
                    "(d p) f -> p d f", d=2))
        wk_t = load_merged("wk", wk, 8)
        wv_t = load_merged("wv", wv, 8)
        msk0 = persist.tile([128, 512], BF16, tag="msk0")
        nc.sync.dma_start(msk0[:], msk.ap()[:, :])
        wo_t = load_merged("wo", wo, 4)

        w_sb = {
            "wq": [wq_t[:, di * FPC:(di + 1) * FPC] for di in range(8)],
            "wk": [wk_t[:, di * FPC:(di + 1) * FPC] for di in range(8)],
            "wv": [wv_t[:, di * FPC:(di + 1) * FPC] for di in range(8)],
        }
        xT_sb = [xT_t[:, di * SEQ:(di + 1) * SEQ] for di in range(8)]
        wo_sb = [wo_t[:, fi * DIM:(fi + 1) * DIM] for fi in range(4)]
        # E[h, f] = 1 iff head-half h owns feature row f: one rank-2 matmul
        # broadcasts both halves' recip rows across their 64 partitions
        emat = persist.tile([33, 128], BF16, tag="emat")
        nc.gpsimd.memset(emat[:], 0.0)
        nc.gpsimd.memset(emat[0:1, 0:DH], 1.0)
        nc.gpsimd.memset(emat[32:33, DH:128], 1.0)
        # persistent sum-row staging: partitions 1..31 stay 1.0 forever so
        # ln/exp over [0:33] yields finite values on the unused rows (they
        # meet only zeros in emat, but NaN*0 would still poison the matmul)
        srow_p = persist.tile([33, 512], F32, tag="srow")
        nc.gpsimd.memset(srow_p[0:32, :], 1.0)

        # ---- stage B: QKV projections -----------------------------------
        # Emission order interleaves per-pair Q/K with V tile groups so the
        # attention stage (ACT exps) can start while QKV still runs on PE.
        qk_sb = {"q": [], "k": []}
        for qn in ("q", "k"):
            for fi in range(4):
                qk_sb[qn].append(
                    persist.tile([128, SEQ], BF16, tag=f"{qn}{fi}",
                                 name=f"{qn}{fi}"))
        v_sb = [persist.tile([128, HPC * VSTRIDE], BF16, tag=f"v{ti}",
                             name=f"v{ti}") for ti in range(NKT)]

        def emit_qk(qn, wn, fi):
            # Q, K in [feature, token] layout (w stationary, xT moving).
            # 4 simultaneous accumulators (2 mm banks + one s-tile's halves)
            # keep each weight stationary across 4 matmuls for LDW dedupe.
            t = qk_sb[qn][fi]
            wt = {"wq": wq_t, "wk": wk_t}[wn]
            st = ps_s.tile([128, 1024], F32, tag="s", name="ch_s")
            ch = [ps_mm.tile([128, 512], F32, tag="mm", name="ch0"),
                  ps_mm.tile([128, 512], F32, tag="mm", name="ch1"),
                  st[:, 0:512], st[:, 512:1024]]
            for di in range(8):
                for tck in range(4):
                    nc.tensor.matmul(
                        ch[tck],
                        wt[:, di * FPC + fi * 128:di * FPC + (fi + 1) * 128],
                        xT_t[:, di * SEQ + tck * 512:di * SEQ + (tck + 1) * 512],
                        start=(di == 0), stop=(di == 7))
            for tck in range(4):
                if tck % 2 == 0:
                    nc.vector.tensor_copy(t[:, tck * 512:(tck + 1) * 512], ch[tck])
                else:
                    nc.scalar.copy(t[:, tck * 512:(tck + 1) * 512], ch[tck])

        def emit_v(ti):
            # V in [token, feature] layout (xT stationary, wv moving), strided
            # into VSTRIDE-blocks with a ones column per head
            t = v_sb[ti]
            p = ps_mm.tile([128, 512], F32, tag="mm", name="p_v")
            for di in range(8):
                nc.tensor.matmul(
                    p[:], xT_t[:, di * SEQ + ti * 128:di * SEQ + (ti + 1) * 128],
                    wv_t[:, di * FPC:(di + 1) * FPC],
                    start=(di == 0), stop=(di == 7))
            dst = t[:].rearrange("p (h c) -> p h c", h=HPC)[:, :, 0:DH]
            src = p[:].rearrange("p (h c) -> p h c", h=HPC)
            nc.vector.tensor_copy(dst, src)
            nc.gpsimd.memset(
                t[:].rearrange("p (h c) -> p h c", h=HPC)[:, :, DH:DH + 1], 1.0)

        ot_sb = [persist.tile([128, SEQ], BF16, tag=f"ot{fi}", name=f"ot{fi}")
                 for fi in range(4)]
        pts_map = {}
        po_map = {}
        rrow_map = {}

        def emit_s_tile(pr, ci, j):
            # S^T strip + exp into a pt tile for (head pair pr, q-chunk ci)
            q0 = ci * 512
            ps = ps_s.tile([128, 1024], F32, tag="s", name="ps_st")
            for half in range(2):   # head A / head B, row-tiled
                nc.tensor.matmul(
                    ps[:, half * 512:(half + 1) * 512],
                    qk_sb["k"][pr][half * 64:(half + 1) * 64,
                                   j * 128:(j + 1) * 128],
                    qk_sb["q"][pr][half * 64:(half + 1) * 64, q0:q0 + 512],
                    start=True, stop=True)
            pt = pt_pool.tile([128, 1024], BF16, tag=f"pt{j}", name="pt",
                              bufs=2)
            pts_map[(pr, ci)].append(pt)
            r = j - 4 * ci
            if r < 0:
                nc.scalar.activation(pt[:], ps[:], Exp, scale=SCALE)
            else:
                # diagonal tile: columns ql >= 128r are valid; the
                # rest must be zero (PV streams the full chunk)
                pt3 = pt[:].rearrange("p (b w) -> p b w", b=2)[:, :, 128 * r:]
                ps3 = ps[:].rearrange("p (b w) -> p b w", b=2)[:, :, 128 * r:]
                if r > 0:
                    nc.gpsimd.memset(
                        pt[:].rearrange("p (b w) -> p b w", b=2)[:, :, 0:128 * r],
                        0.0)
                nc.scalar.activation(pt3, ps3, Exp, scale=SCALE)
                m3 = msk0[:, :512 - 128 * r][:, None, :].broadcast_to(
                    [128, 2, 512 - 128 * r])
                # mask-mul on gpsimd keeps the DVE queue out of PV's pt deps
                nc.gpsimd.tensor_mul(pt3, pt3, m3)

        def emit_pv_step(pr, ci, j):
            # one k-tile of both halves' PV accumulation chains
            nj = 4 * ci + 4
            if j == 0:
                po_map[(pr, ci)] = [
                    ps_o.tile([DH + 1, 512], F32, tag="o", name="po")
                    for _ in range(2)]
            for half in range(2):
                h = 2 * pr + half
                nc.tensor.matmul(
                    po_map[(pr, ci)][half][:],
                    v_sb[j][:, h * VSTRIDE:h * VSTRIDE + DH + 1],
                    pts_map[(pr, ci)][j][:, half * 512:(half + 1) * 512],
                    start=(j == 0), stop=(j == nj - 1))

        def emit_recip(pr, ci):
            # gather both halves' row sums into one [2,512] tile, one LN +
            # one EXP; consumed by emit_div one phase later
            for half in range(2):
                nc.vector.tensor_copy(
                    srow_p[32 * half:32 * half + 1, :],
                    po_map[(pr, ci)][half][DH:DH + 1, :])
            rrow2 = work.tile([33, 512], BF16, tag="rrow", name="rrow")
            _act_recip2(nc, work, rrow2[:], srow_p[:])
            rrow_map[(pr, ci)] = rrow2
            pts_map.pop((pr, ci))

        def emit_div(pr, ci):
            # broadcast both recip rows across their 64-partition halves with
            # one rank-2 matmul and scale O^T into ot_sb
            q0 = ci * 512
            fi = pr
            rb_ps = ps_mm.tile([128, 512], F32, tag="mm", name="rb_ps")
            nc.tensor.matmul(rb_ps[:], emat[:], rrow_map[(pr, ci)][:],
                             start=True, stop=True)
            rb = work.tile([128, 512], BF16, tag="rb", name="rb")
            nc.vector.tensor_copy(rb[:], rb_ps[:])
            for half in range(2):
                row = half * 64
                po = po_map[(pr, ci)][half]
                nc.vector.tensor_mul(
                    ot_sb[fi][row:row + 64, q0:q0 + 512],
                    po[0:DH, :], rb[row:row + 64, :])
            po_map.pop((pr, ci))
            rrow_map.pop((pr, ci))

        def emit_phase(s_pair, pv_pair, pending_div=None):
            # Interleave S tiles of s_pair with PV chain steps of pv_pair at
            # k-tile granularity: the PE stream stays dense while ACT's exp
            # queue is consumed one phase after it is filled. PV trails the
            # S stream by LEAD tiles so pending_div's rb matmul (which gates
            # PV's PSUM buffers and itself waits on the ACT reciprocal) has
            # PE work queued ahead of it.
            sj = 4 * s_pair[1] + 4 if s_pair else 0
            pj = 4 * pv_pair[1] + 4 if pv_pair else 0
            lead = 5 if (s_pair and pv_pair) else 0
            if s_pair:
                pts_map[s_pair] = []
            if pending_div and lead == 0:
                emit_div(*pending_div)
            for jj in range(max(sj, pj + lead if pv_pair else 0)):
                if s_pair and jj < sj:
                    emit_s_tile(s_pair[0], s_pair[1], jj)
                if jj == lead and pending_div and lead > 0:
                    emit_div(*pending_div)
                if pv_pair and 0 <= jj - lead < pj:
                    emit_pv_step(pv_pair[0], pv_pair[1], jj - lead)
            if pv_pair:
                emit_recip(*pv_pair)

        def emit_proj(ci):
            # projection for chunk ci's columns (all pairs' OT rows ready)
            for ei in range(8):
                p = ps_mm.tile([128, 512], F32, tag="mm", name="p_proj")
                for fi in range(4):
                    nc.tensor.matmul(
                        p[:], wo_t[:, fi * DIM + ei * 128:fi * DIM + (ei + 1) * 128],
                        ot_sb[fi][:, ci * 512:(ci + 1) * 512],
                        start=(fi == 0), stop=(fi == 3))
                os_ = work.tile([128, 512], BF16, tag="os", name="os")
                nc.vector.tensor_copy(os_[:], p[:])
                nc.sync.dma_start(
                    outT.ap()[ei * 128:(ei + 1) * 128,
                              ci * 512:(ci + 1) * 512], os_[:])

        # Schedule: each phase runs pair pr's S tiles interleaved with pair
        # pr-1's PV chain (k-tile granular, PV trailing by 3 tiles). exp(pr)
        # completes during the phase that produces it, one phase before
        # PV(pr) consumes it; the reciprocal queued at a phase's end is
        # consumed (emit_div) a lead into the NEXT phase — the PE never
        # waits on ACT. Q/K, V and the output projection fill the remaining
        # slots (and give ACT time to catch up on its exp backlog).
        emit_qk("q", "wq", 0)
        emit_qk("k", "wk", 0)
        emit_phase((0, 0), None)
        emit_qk("q", "wq", 1)
        emit_qk("k", "wk", 1)
        for ti in range(4):
            emit_v(ti)
        emit_phase((1, 0), (0, 0))
        emit_qk("q", "wq", 2)
        emit_qk("k", "wk", 2)
        emit_phase((2, 0), (1, 0), pending_div=(0, 0))
        emit_qk("q", "wq", 3)
        emit_qk("k", "wk", 3)
        emit_phase((3, 0), (2, 0), pending_div=(1, 0))
        for ci in range(1, NQC):
            emit_phase((0, ci), (3, ci - 1), pending_div=(2, ci - 1))
            for ti in range(4 * ci, 4 * ci + 4):
                emit_v(ti)
            emit_div(3, ci - 1)
            emit_proj(ci - 1)
            emit_phase((1, ci), (0, ci))
            emit_phase((2, ci), (1, ci), pending_div=(0, ci))
            emit_phase((3, ci), (2, ci), pending_div=(1, ci))
        emit_phase(None, (3, NQC - 1), pending_div=(2, NQC - 1))
        emit_div(3, NQC - 1)
        emit_proj(NQC - 1)
    return nc


_NC = None


def _get_nc():
    global _NC
    if _NC is None:
        _patch_walrus_wait_limit()
        _NC = build_kernel()
    return _NC


def _host_masks():
    kl = np.arange(128)[:, None]
    ql = np.arange(512)[None, :]
    return (kl <= ql).astype(np.float32).astype(ml_dtypes.bfloat16)


def kernel(x, w_qkv, w_out, _trace=False, _trace_kwargs=None):
    x = np.asarray(x, dtype=np.float32)
    w_qkv = np.asarray(w_qkv, dtype=np.float32)
    w_out = np.asarray(w_out, dtype=np.float32)
    nc = _get_nc()

    msk = _host_masks()
    in_maps = []
    for c in range(NCORES):
        b, g = c // 2, c % 2
        cols = slice(g * FPC, (g + 1) * FPC)
        in_maps.append({
            "xT": np.ascontiguousarray(x[b].T).astype(ml_dtypes.bfloat16),
            "wq": w_qkv[:, 0 * DIM:1 * DIM][:, cols].astype(ml_dtypes.bfloat16),
            "wk": w_qkv[:, 1 * DIM:2 * DIM][:, cols].astype(ml_dtypes.bfloat16),
            "wv": w_qkv[:, 2 * DIM:3 * DIM][:, cols].astype(ml_dtypes.bfloat16),
            "wo": w_out[g * FPC:(g + 1) * FPC, :].astype(ml_dtypes.bfloat16),
            "msk": msk,
        })

    res = run_bass_kernel_spmd(
        nc, in_maps, core_ids=list(range(NCORES)),
        trace=_trace, **(_trace_kwargs or {}))
    out = np.empty((4, SEQ, DIM), dtype=np.float32)
    for b in range(4):
        out[b] = (res.results[2 * b]["outT"].astype(np.float32)
                  + res.results[2 * b + 1]["outT"].astype(np.float32)).T
    if _trace:
        kernel.last_results = res
    return out

